# revision 1
# baseline (speedup 1.0000x reference)
"""Bass/Trainium2 kernel for nn_HardAndLayer.

Reference computation:
    out[o] = AND_i ( x[i] OR NOT w[o,i] )   , w in {0.0, 1.0}, x bool
           = NOT any_i ( w[o,i] AND NOT x[i] )

Strategy:
  - Weights are binary -> bit-pack 32 weights per uint32 word on the host.
    Full W [8192, 8192] f32 (256 MB) becomes [8192, 256] uint32 (8 MB).
  - Shard rows across 8 cores (1024 rows / core = 1 MB packed per core).
    The bit-packed NOT-x vector is replicated across the 128 partitions and
    prepended to the per-core weight buffer, so a single DMA stream feeds
    everything.
  - Per core: 3 chunked DMAs (issued on both HWDGE engines: sync + scalar),
    8 VectorE bitwise_and ops (one per 128-row tile), 8 ScalarE
    activation(Copy)+accumulate ops producing the per-row sum of AND words.
    A row has a violation iff its sum > 0.
  - Host: out[row] = (sum == 0).
"""

import sys

if "/opt/trn_rl_repo" not in sys.path:
    sys.path.insert(0, "/opt/trn_rl_repo")

import numpy as np

import concourse.bacc as bacc
import concourse.bass as bass
import concourse.mybir as mybir
import concourse.tile as tile
from concourse.bass_utils import run_bass_kernel_spmd

OUT, IN = 8192, 8192
NCORES = 8
P = 128                 # SBUF partitions
NW = IN // 32           # uint32 words per row = 256
RPC = OUT // NCORES     # rows per core = 1024
NT = RPC // P           # 128-row tiles per core = 8
TOT = NW * (NT + 1)     # words per partition incl. leading nx block = 2304

# chunk boundaries in tiles; chunk 0 additionally carries the leading nx block
CHUNKS = [(0, 1), (1, 4), (4, 8)]

_cached = {}


def _build_module():
    nc = bacc.Bacc(
        None,
        enable_partition_id=False,
        enable_asserts=False,
        monotonic_sem_count=0,
    )
    wx = nc.dram_tensor("wx", [P, TOT], mybir.dt.uint32, kind="ExternalInput")
    out = nc.dram_tensor("out", [P, NT], mybir.dt.float32, kind="ExternalOutput")

    with tile.TileContext(nc) as tc:
        with tc.tile_pool(name="sbuf", bufs=1) as pool:
            # chunk tiles; chunk 0 carries [nx | tile0]
            ctiles = []
            for ci, (ta, tb) in enumerate(CHUNKS):
                lo = (ta + 1) * NW if ci else 0
                hi = (tb + 1) * NW
                ck = pool.tile([P, hi - lo], mybir.dt.uint32, tag=f"c{ci}")
                eng = nc.sync if ci % 2 == 0 else nc.scalar
                eng.dma_start(ck[:], wx[:, lo:hi])
                ctiles.append((ck, lo))

            nxs = ctiles[0][0][:, 0:NW]
            res = pool.tile([P, NT], mybir.dt.float32)

            def nx_bcast(k):
                """nx AP broadcast k times along a stride-0 middle dim."""
                ap = nxs
                return bass.AP(
                    ap.tensor,
                    ap.offset,
                    [list(ap.ap[0])] + [[0, k]] + [list(p) for p in ap.ap[1:]],
                )

            # Per-tile ANDs for tiles 0-3 (feeds the ScalarE reduce chain
            # early); one merged 4-tile AND for chunk 2 (tiles 4-7).
            # Reduce split: ScalarE activation+accum for tiles 0-4, one
            # merged 3-tile VectorE tensor_reduce for tiles 5-7.
            for t in range(4):
                ci = next(i for i, (ta, tb) in enumerate(CHUNKS) if ta <= t < tb)
                ck, lo = ctiles[ci]
                off = (t + 1) * NW - lo
                sl = ck[:, off : off + NW]
                nc.vector.tensor_tensor(
                    out=sl, in0=sl, in1=nxs, op=mybir.AluOpType.bitwise_and
                )
                nc.scalar.activation(
                    out=sl,
                    in_=sl,
                    func=mybir.ActivationFunctionType.Copy,
                    accum_out=res[:, t : t + 1],
                )
            ck2, lo2 = ctiles[2]
            c2all = ck2[:].rearrange("p (t v) -> p t v", v=NW)
            nc.vector.tensor_tensor(
                out=c2all, in0=c2all, in1=nx_bcast(4), op=mybir.AluOpType.bitwise_and
            )
            nc.scalar.activation(
                out=ck2[:, 0:NW],
                in_=ck2[:, 0:NW],
                func=mybir.ActivationFunctionType.Copy,
                accum_out=res[:, 4:5],
            )
            nc.vector.tensor_reduce(
                out=res[:, 5:8],
                in_=ck2[:, NW : 4 * NW].rearrange("p (t v) -> p t v", v=NW),
                axis=mybir.AxisListType.X,
                op=mybir.AluOpType.max,
            )

            nc.sync.dma_start(out[:], res[:])
    nc.compile()
    return nc


def _pack_bits(bool2d: np.ndarray) -> np.ndarray:
    """[N, 8192] bool -> [N, 256] uint32 (consistent bit order)."""
    u8 = np.packbits(bool2d, axis=-1, bitorder="little")
    return u8.view(np.uint32)


def kernel(weights: np.ndarray, x: np.ndarray, **run_kwargs):
    wbits = _pack_bits(np.asarray(weights) != 0)                # [8192, 256]
    nxbits = _pack_bits((~np.asarray(x, dtype=bool))[None, :])  # [1, 256]
    nx_rep = np.broadcast_to(nxbits, (P, NW))

    in_maps = []
    for c in range(NCORES):
        wr = (
            wbits[c * RPC : (c + 1) * RPC]
            .reshape(NT, P, NW)
            .transpose(1, 0, 2)
            .reshape(P, NT * NW)
        )
        in_maps.append({"wx": np.ascontiguousarray(np.concatenate([nx_rep, wr], axis=1))})

    if "nc" not in _cached:
        _cached["nc"] = _build_module()
    nc = _cached["nc"]

    r = run_bass_kernel_spmd(nc, in_maps, core_ids=list(range(NCORES)), **run_kwargs)

    outs = []
    for c in range(NCORES):
        m = r.results[c]["out"]            # [P, NT] f32, m[p, t] = sum of AND words
        outs.append(m.T.reshape(RPC))      # row t*128+p within core
    sums = np.concatenate(outs)            # [8192]
    result = sums == 0.0
    if run_kwargs:
        return result, r
    return result



# revision 7
# speedup vs baseline: 1.0315x; 1.0315x over previous
"""Bass/Trainium2 kernel for nn_HardAndLayer.

Reference computation:
    out[o] = AND_i ( x[i] OR NOT w[o,i] )   , w in {0.0, 1.0}, x bool
           = NOT any_i ( w[o,i] AND NOT x[i] )

Strategy:
  - Weights are binary -> bit-pack 32 weights per uint32 word on the host.
    Full W [8192, 8192] f32 (256 MB) becomes [8192, 256] uint32 (8 MB).
  - Shard rows across 8 cores (1024 rows / core = 1 MB packed per core).
    The bit-packed NOT-x vector is replicated across the 128 partitions and
    prepended to the per-core weight buffer.
  - DMA split across 3 queues: sync HWDGE, scalar HWDGE and gpsimd SWDGE
    so transfers run in parallel and compute starts on early chunks.
  - Compute (bitwise ops exist only on DVE; the HW reduce-folding path is
    arithmetic-only, so AND + reduce are separate passes): DVE does the
    bitwise ANDs (merged over tile groups where it helps), the per-tile
    "any word nonzero" reduce is split between ScalarE
    activation-accumulate and a merged DVE max-reduce.
  - The output DMA is issued OUTSIDE the TileContext so its HBM-write
    receipt overlaps the fixed end-of-NEFF semaphore-clear storm instead
    of preceding it.
  - Host: out[row] = (reduced word == 0).
"""

import sys

if "/opt/trn_rl_repo" not in sys.path:
    sys.path.insert(0, "/opt/trn_rl_repo")

import numpy as np

import concourse.bacc as bacc
import concourse.bass as bass
import concourse.mybir as mybir
import concourse.tile as tile
from concourse.bass_utils import run_bass_kernel_spmd

OUT, IN = 8192, 8192
NCORES = 8
P = 128                 # SBUF partitions
NW = IN // 32           # uint32 words per row = 256
RPC = OUT // NCORES     # rows per core = 1024
NT = RPC // P           # 128-row tiles per core = 8
TOT = NW * (NT + 1)     # words per partition incl. leading nx block = 2304

# (engine, word_lo, word_hi) DMA chunks; layout is [nx | t0 | t1 | ... | t7]
DMA_CHUNKS = [
    ("sync", 0, 2 * NW),            # nx | t0
    ("scalar", 2 * NW, 4 * NW),     # t1 t2
    ("sync", 4 * NW, 6 * NW),       # t3 t4
    ("scalar", 6 * NW, 8 * NW),     # t5 t6
    ("gpsimd", 8 * NW, 9 * NW),     # t7
]
# AND groups (consecutive tiles merged into one DVE op), in emission order.
AND_GROUPS = [(0, 1), (1, 3), (3, 5), (7, 8), (5, 7)]
# Reduce assignment: ScalarE activation-accumulate vs merged DVE max-reduce.
SCALAR_REDUCE_TILES = [0, 1, 2, 3, 7]
DVE_REDUCE_GROUP = (4, 7)           # one merged [p, 3, NW] max-reduce

_cached = {}


def _build_module():
    nc = bacc.Bacc(
        None,
        enable_partition_id=False,
        enable_asserts=False,
        monotonic_sem_count=0,
    )
    wx = nc.dram_tensor("wx", [P, TOT], mybir.dt.uint32, kind="ExternalInput")
    out = nc.dram_tensor("out", [P, NT], mybir.dt.uint32, kind="ExternalOutput")

    # Plain SBUF tensor (concrete address) so the post-TileContext output
    # DMA can reference it without symbolic-AP lowering.
    res_t = nc.alloc_sbuf_tensor("res", [P, NT], mybir.dt.uint32)

    with tile.TileContext(nc) as tc:
        with tc.tile_pool(name="sbuf", bufs=1) as pool:
            wbuf = pool.tile([P, TOT], mybir.dt.uint32)

            for eng_name, lo, hi in DMA_CHUNKS:
                eng = getattr(nc, eng_name)
                eng.dma_start(wbuf[:, lo:hi], wx[:, lo:hi])

            nxs = wbuf[:, 0:NW]

            def nx_bcast(k):
                """nx AP broadcast k times along a stride-0 middle dim."""
                ap = nxs
                return bass.AP(
                    ap.tensor,
                    ap.offset,
                    [list(ap.ap[0])] + [[0, k]] + [list(p) for p in ap.ap[1:]],
                )

            def group_ap(a, b):
                """Tiles [a, b) as a [P, b-a, NW] view of wbuf."""
                return wbuf[:, (a + 1) * NW : (b + 1) * NW].rearrange(
                    "p (t v) -> p t v", v=NW
                )

            def tslice(t):
                off = (t + 1) * NW
                return wbuf[:, off : off + NW]

            scalar_done = set()
            for a, b in AND_GROUPS:
                if b - a == 1:
                    sl = tslice(a)
                    nc.vector.tensor_tensor(
                        out=sl, in0=sl, in1=nxs, op=mybir.AluOpType.bitwise_and
                    )
                else:
                    g = group_ap(a, b)
                    nc.vector.tensor_tensor(
                        out=g, in0=g, in1=nx_bcast(b - a),
                        op=mybir.AluOpType.bitwise_and,
                    )
                # Emit the scalar reduces for this group right away so the
                # scheduler lets ScalarE trail the DVE AND stream tile by
                # tile.
                for t in range(a, b):
                    if t in SCALAR_REDUCE_TILES:
                        sl = tslice(t)
                        nc.scalar.activation(
                            out=sl,
                            in_=sl,
                            func=mybir.ActivationFunctionType.Copy,
                            accum_out=res_t[:, t : t + 1].bitcast(
                                mybir.dt.float32
                            ),
                        )
                        scalar_done.add(t)

            ra, rb = DVE_REDUCE_GROUP
            nc.vector.tensor_reduce(
                out=res_t[:, ra:rb],
                in_=group_ap(ra, rb),
                axis=mybir.AxisListType.X,
                op=mybir.AluOpType.max,
            )

    # Issued after the TileContext end-barrier: all compute is done, and the
    # DMA's completion receipt overlaps the NEFF's semaphore-clear epilogue.
    # The completion semaphore is required by codegen but never waited on —
    # the NEFF-exit drain guarantees the queue is flushed before host reads.
    out_sem = nc.alloc_semaphore("out_sem")
    nc.sync.dma_start(out[:], res_t[:]).then_inc(out_sem, 16)
    nc.compile()
    return nc


def _pack_bits(bool2d: np.ndarray) -> np.ndarray:
    """[N, 8192] bool -> [N, 256] uint32 (consistent bit order)."""
    u8 = np.packbits(bool2d, axis=-1, bitorder="little")
    return u8.view(np.uint32)


def kernel(weights: np.ndarray, x: np.ndarray, **run_kwargs):
    wbits = _pack_bits(np.asarray(weights) != 0)                # [8192, 256]
    nxbits = _pack_bits((~np.asarray(x, dtype=bool))[None, :])  # [1, 256]
    nx_rep = np.broadcast_to(nxbits, (P, NW))

    in_maps = []
    for c in range(NCORES):
        wr = (
            wbits[c * RPC : (c + 1) * RPC]
            .reshape(NT, P, NW)
            .transpose(1, 0, 2)
            .reshape(P, NT * NW)
        )
        in_maps.append({"wx": np.ascontiguousarray(np.concatenate([nx_rep, wr], axis=1))})

    if "nc" not in _cached:
        _cached["nc"] = _build_module()
    nc = _cached["nc"]

    r = run_bass_kernel_spmd(nc, in_maps, core_ids=list(range(NCORES)), **run_kwargs)

    outs = []
    for c in range(NCORES):
        m = r.results[c]["out"]            # [P, NT] uint32, nonzero = violation
        outs.append(m.T.reshape(RPC))      # row t*128+p within core
    vio = np.concatenate(outs)             # [8192]
    result = vio == 0
    if run_kwargs:
        return result, r
    return result


# revision 10
# speedup vs baseline: 1.5239x; 1.4773x over previous
"""Bass/Trainium2 kernel for nn_HardAndLayer.

Reference computation:
    out[o] = AND_i ( x[i] OR NOT w[o,i] )   , w in {0.0, 1.0}, x bool
           = NOT any_i ( w[o,i] AND NOT x[i] )

Strategy:
  - Weights are binary -> bit-pack 32 weights per uint32 word on the host.
    Full W [8192, 8192] f32 (256 MB) becomes [8192, 256] uint32 (8 MB).
  - Shard rows across 8 cores (1024 rows / core = 1 MB packed per core).
    The bit-packed NOT-x vector is replicated across the 128 partitions and
    prepended to the per-core weight buffer.
  - The input DMA is hoisted to the top of the sync engine's instruction
    stream (right after its preamble), so the whole 1.15 MB transfer runs
    during NEFF engine boot / instruction fetch, overlapped with framework
    startup, and is complete (or nearly so) by the time the compute
    engines come out of the startup barrier.
  - Compute (bitwise ops exist only on DVE; the HW reduce-folding path is
    arithmetic-only, so AND + reduce are separate passes): DVE does the
    bitwise ANDs merged over tile groups, the per-tile "any word nonzero"
    reduce is split between ScalarE activation-accumulate and a merged DVE
    max-reduce.
  - The framework's const-AP memsets are deleted (nothing in this kernel
    consumes the const APs); they would otherwise sit on the GpSimd stream
    ahead of everything else.
  - The output DMA is issued OUTSIDE the TileContext so its HBM-write
    receipt overlaps the fixed end-of-NEFF semaphore-clear epilogue
    instead of preceding it.
  - Host: out[row] = (reduced word == 0).
"""

import sys

if "/opt/trn_rl_repo" not in sys.path:
    sys.path.insert(0, "/opt/trn_rl_repo")

import numpy as np

import concourse.bacc as bacc
import concourse.bass as bass
import concourse.mybir as mybir
import concourse.tile as tile
from concourse.bass_utils import run_bass_kernel_spmd

OUT, IN = 8192, 8192
NCORES = 8
P = 128                 # SBUF partitions
NW = IN // 32           # uint32 words per row = 256
RPC = OUT // NCORES     # rows per core = 1024
NT = RPC // P           # 128-row tiles per core = 8
TOT = NW * (NT + 1)     # words per partition incl. leading nx block = 2304

# DVE bitwise-AND groups (tile ranges merged into one op each).
AND_GROUPS = [(0, 2), (2, 6), (6, 8)]
# Reduce split: ScalarE activation-accumulate tiles vs merged DVE max-reduce.
SCALAR_REDUCE_TILES = [0, 1, 2, 3]
DVE_REDUCE_GROUP = (4, 8)

_cached = {}


def _build_module():
    nc = bacc.Bacc(
        None,
        enable_partition_id=False,
        enable_asserts=False,
        monotonic_sem_count=0,
    )
    wx = nc.dram_tensor("wx", [P, TOT], mybir.dt.uint32, kind="ExternalInput")
    out = nc.dram_tensor("out", [P, NT], mybir.dt.uint32, kind="ExternalOutput")

    # Plain SBUF tensors (concrete addresses) so instructions outside the
    # TileContext can reference them without symbolic-AP lowering.
    wbuf_t = nc.alloc_sbuf_tensor("wbuf", [P, TOT], mybir.dt.uint32)
    res_t = nc.alloc_sbuf_tensor("res", [P, NT], mybir.dt.uint32)

    main_bb = nc.main_func.blocks[0]
    insts = main_bb.instructions

    # --- Early input DMA -------------------------------------------------
    # Emit the input DMA, then hoist it to right after the sync engine's
    # preamble so the transfer overlaps NEFF boot.
    in_sem = nc.alloc_semaphore("in_sem")
    nc.sync.dma_start(wbuf_t[:], wx[:]).then_inc(in_sem, 16)
    dma_inst = insts[-1]
    assert isinstance(dma_inst, mybir.InstDMACopy), type(dma_inst)
    insts.remove(dma_inst)
    anchor = nc.sync.preamble_end
    insts.insert(insts.index(anchor) + 1, dma_inst)

    # --- Drop the framework's const-AP memsets ---------------------------
    # They are the first "useful" instructions in the NEFF and nothing in
    # this kernel reads the const APs (activation uses func=Copy whose bias
    # stays an immediate).
    for i in [
        i
        for i in insts
        if isinstance(i, mybir.InstMemset) and "const-" in i.concise()
    ]:
        insts.remove(i)

    # Wait for the boot-time input DMA before the compute engines enter the
    # TileContext body (emitted outside the context so its scheduling sim
    # never sees a wait it cannot satisfy).
    nc.vector.wait_ge(in_sem, 16)
    nc.scalar.wait_ge(in_sem, 16)

    with tile.TileContext(nc) as tc:
        nxs = wbuf_t[:, 0:NW]

        def nx_bcast(k):
            """nx AP broadcast k times along a stride-0 middle dim."""
            ap = nxs
            return bass.AP(
                ap.tensor,
                ap.offset,
                [list(ap.ap[0])] + [[0, k]] + [list(p) for p in ap.ap[1:]],
            )

        def group_ap(a, b):
            """Tiles [a, b) as a [P, b-a, NW] view of wbuf."""
            return wbuf_t[:, (a + 1) * NW : (b + 1) * NW].rearrange(
                "p (t v) -> p t v", v=NW
            )

        def tslice(t):
            off = (t + 1) * NW
            return wbuf_t[:, off : off + NW]

        for a, b in AND_GROUPS:
            g = group_ap(a, b)
            nc.vector.tensor_tensor(
                out=g, in0=g, in1=nx_bcast(b - a), op=mybir.AluOpType.bitwise_and
            )
            for t in range(a, b):
                if t in SCALAR_REDUCE_TILES:
                    sl = tslice(t)
                    nc.scalar.activation(
                        out=sl,
                        in_=sl,
                        func=mybir.ActivationFunctionType.Copy,
                        accum_out=res_t[:, t : t + 1].bitcast(mybir.dt.float32),
                    )

        ra, rb = DVE_REDUCE_GROUP
        nc.vector.tensor_reduce(
            out=res_t[:, ra:rb],
            in_=group_ap(ra, rb),
            axis=mybir.AxisListType.X,
            op=mybir.AluOpType.max,
        )

    # Issued after the TileContext end-barrier: all compute is done, and the
    # DMA's completion receipt overlaps the NEFF's semaphore-clear epilogue.
    # The completion semaphore is required by codegen but never waited on —
    # the NEFF-exit drain guarantees the queue is flushed before host reads.
    out_sem = nc.alloc_semaphore("out_sem")
    nc.sync.dma_start(out[:], res_t[:]).then_inc(out_sem, 16)
    nc.compile()
    return nc


def _pack_bits(bool2d: np.ndarray) -> np.ndarray:
    """[N, 8192] bool -> [N, 256] uint32 (consistent bit order)."""
    u8 = np.packbits(bool2d, axis=-1, bitorder="little")
    return u8.view(np.uint32)


def kernel(weights: np.ndarray, x: np.ndarray, **run_kwargs):
    wbits = _pack_bits(np.asarray(weights) != 0)                # [8192, 256]
    nxbits = _pack_bits((~np.asarray(x, dtype=bool))[None, :])  # [1, 256]
    nx_rep = np.broadcast_to(nxbits, (P, NW))

    in_maps = []
    for c in range(NCORES):
        wr = (
            wbits[c * RPC : (c + 1) * RPC]
            .reshape(NT, P, NW)
            .transpose(1, 0, 2)
            .reshape(P, NT * NW)
        )
        in_maps.append({"wx": np.ascontiguousarray(np.concatenate([nx_rep, wr], axis=1))})

    if "nc" not in _cached:
        _cached["nc"] = _build_module()
    nc = _cached["nc"]

    r = run_bass_kernel_spmd(nc, in_maps, core_ids=list(range(NCORES)), **run_kwargs)

    outs = []
    for c in range(NCORES):
        m = r.results[c]["out"]            # [P, NT] uint32, nonzero = violation
        outs.append(m.T.reshape(RPC))      # row t*128+p within core
    vio = np.concatenate(outs)             # [8192]
    result = vio == 0
    if run_kwargs:
        return result, r
    return result


# revision 12
# speedup vs baseline: 1.6166x; 1.0608x over previous
"""Bass/Trainium2 kernel for nn_HardAndLayer.

Reference computation:
    out[o] = AND_i ( x[i] OR NOT w[o,i] )   , w in {0.0, 1.0}, x bool
           = NOT any_i ( w[o,i] AND NOT x[i] )

Strategy:
  - Weights are binary -> bit-pack 32 weights per uint32 word on the host.
    Full W [8192, 8192] f32 (256 MB) becomes [8192, 256] uint32 (8 MB).
  - Shard rows across 8 cores (1024 rows / core = 1 MB packed per core).
    The bit-packed NOT-x vector is replicated across the 128 partitions and
    prepended to the per-core weight buffer.
  - The input DMA is hoisted to the top of the sync engine's instruction
    stream (right after its preamble), so the whole 1.15 MB transfer runs
    during NEFF engine boot / instruction fetch, overlapped with framework
    startup, and is complete (or nearly so) by the time the compute
    engines come out of the startup barrier.
  - Compute (bitwise ops exist only on DVE; the HW reduce-folding path is
    arithmetic-only, so AND + reduce are separate passes): DVE does the
    bitwise ANDs merged over tile groups, the per-tile "any word nonzero"
    reduce is split between ScalarE activation-accumulate and a merged DVE
    max-reduce.
  - The framework's const-AP memsets are deleted (nothing in this kernel
    consumes the const APs); they would otherwise sit on the GpSimd stream
    ahead of everything else.
  - The output DMA is issued OUTSIDE the TileContext so its HBM-write
    receipt overlaps the fixed end-of-NEFF semaphore-clear epilogue
    instead of preceding it.
  - Host: out[row] = (reduced word == 0).
"""

import sys

if "/opt/trn_rl_repo" not in sys.path:
    sys.path.insert(0, "/opt/trn_rl_repo")

import numpy as np

import concourse.bacc as bacc
import concourse.bass as bass
import concourse.mybir as mybir
import concourse.tile as tile
from concourse.bass_utils import run_bass_kernel_spmd

OUT, IN = 8192, 8192
NCORES = 8
P = 128                 # SBUF partitions
NW = IN // 32           # uint32 words per row = 256
RPC = OUT // NCORES     # rows per core = 1024
NT = RPC // P           # 128-row tiles per core = 8
TOT = NW * (NT + 1)     # words per partition incl. leading nx block = 2304

# DVE bitwise-AND groups (tile ranges merged into one op each).
AND_GROUPS = [(0, 2), (2, 6), (6, 8)]
# Reduce split: ScalarE activation-accumulate tiles vs merged DVE max-reduce.
SCALAR_REDUCE_TILES = [0, 1, 2, 3]
DVE_REDUCE_GROUP = (4, 8)

_cached = {}


def _build_module():
    nc = bacc.Bacc(
        None,
        enable_partition_id=False,
        enable_asserts=False,
        monotonic_sem_count=0,
    )
    wx = nc.dram_tensor("wx", [P, TOT], mybir.dt.uint32, kind="ExternalInput")
    out = nc.dram_tensor("out", [P, NT], mybir.dt.uint32, kind="ExternalOutput")

    # Plain SBUF tensors (concrete addresses) so instructions outside the
    # TileContext can reference them without symbolic-AP lowering.
    wbuf_t = nc.alloc_sbuf_tensor("wbuf", [P, TOT], mybir.dt.uint32)
    res_t = nc.alloc_sbuf_tensor("res", [P, NT], mybir.dt.uint32)

    main_bb = nc.main_func.blocks[0]
    insts = main_bb.instructions

    # --- Early input DMA -------------------------------------------------
    # Emit the input DMA, then hoist it to right after the sync engine's
    # preamble so the transfer overlaps NEFF boot.
    in_sem = nc.alloc_semaphore("in_sem")
    nc.sync.dma_start(wbuf_t[:], wx[:]).then_inc(in_sem, 16)
    dma_inst = insts[-1]
    assert isinstance(dma_inst, mybir.InstDMACopy), type(dma_inst)
    insts.remove(dma_inst)
    anchor = nc.sync.preamble_end
    insts.insert(insts.index(anchor) + 1, dma_inst)

    # --- Drop the framework's const-AP memsets ---------------------------
    # They are the first "useful" instructions in the NEFF and nothing in
    # this kernel reads the const APs (activation uses func=Copy whose bias
    # stays an immediate).
    for i in [
        i
        for i in insts
        if isinstance(i, mybir.InstMemset) and "const-" in i.concise()
    ]:
        insts.remove(i)

    # Wait for the boot-time input DMA before the compute engines enter the
    # TileContext body (emitted outside the context so its scheduling sim
    # never sees a wait it cannot satisfy). Only the vector engine needs it:
    # ScalarE reads nothing until DVE's AND output exists (tile-tracked), so
    # leaving it unguarded lets its ACT_TABLE_LOAD run during boot.
    nc.vector.wait_ge(in_sem, 16)

    with tile.TileContext(nc) as tc:
        nxs = wbuf_t[:, 0:NW]

        def nx_bcast(k):
            """nx AP broadcast k times along a stride-0 middle dim."""
            ap = nxs
            return bass.AP(
                ap.tensor,
                ap.offset,
                [list(ap.ap[0])] + [[0, k]] + [list(p) for p in ap.ap[1:]],
            )

        def group_ap(a, b):
            """Tiles [a, b) as a [P, b-a, NW] view of wbuf."""
            return wbuf_t[:, (a + 1) * NW : (b + 1) * NW].rearrange(
                "p (t v) -> p t v", v=NW
            )

        def tslice(t):
            off = (t + 1) * NW
            return wbuf_t[:, off : off + NW]

        for a, b in AND_GROUPS:
            g = group_ap(a, b)
            nc.vector.tensor_tensor(
                out=g, in0=g, in1=nx_bcast(b - a), op=mybir.AluOpType.bitwise_and
            )
            for t in range(a, b):
                if t in SCALAR_REDUCE_TILES:
                    sl = tslice(t)
                    nc.scalar.activation(
                        out=sl,
                        in_=sl,
                        func=mybir.ActivationFunctionType.Copy,
                        accum_out=res_t[:, t : t + 1].bitcast(mybir.dt.float32),
                    )

        ra, rb = DVE_REDUCE_GROUP
        nc.vector.tensor_reduce(
            out=res_t[:, ra:rb],
            in_=group_ap(ra, rb),
            axis=mybir.AxisListType.X,
            op=mybir.AluOpType.max,
        )

    # Output DMA: emitted after the TileContext, then hoisted to right after
    # the sync engine's end-of-context drain (which already waits for all
    # compute via the tile global clock). That skips the two exit barriers
    # before the enqueue, and the DMA's completion receipt overlaps the
    # NEFF's semaphore-clear epilogue. The completion semaphore is required
    # by codegen but never waited on — the NEFF-exit drain guarantees the
    # queue is flushed before host reads.
    out_sem = nc.alloc_semaphore("out_sem")
    nc.sync.dma_start(out[:], res_t[:]).then_inc(out_sem, 16)

    end_bb = next(
        bb for bb in nc.main_func.blocks if bb.name.endswith("_end")
    )
    einsts = end_bb.instructions
    out_inst = einsts[-1]
    assert isinstance(out_inst, mybir.InstDMACopy), type(out_inst)
    first_drain = next(
        i
        for i in einsts
        if isinstance(i, mybir.InstDrain) and i.engine == mybir.EngineType.SP
    )
    einsts.remove(out_inst)
    einsts.insert(einsts.index(first_drain) + 1, out_inst)

    nc.compile()
    return nc


def _pack_bits(bool2d: np.ndarray) -> np.ndarray:
    """[N, 8192] bool -> [N, 256] uint32 (consistent bit order)."""
    u8 = np.packbits(bool2d, axis=-1, bitorder="little")
    return u8.view(np.uint32)


def kernel(weights: np.ndarray, x: np.ndarray, **run_kwargs):
    wbits = _pack_bits(np.asarray(weights) != 0)                # [8192, 256]
    nxbits = _pack_bits((~np.asarray(x, dtype=bool))[None, :])  # [1, 256]
    nx_rep = np.broadcast_to(nxbits, (P, NW))

    in_maps = []
    for c in range(NCORES):
        wr = (
            wbits[c * RPC : (c + 1) * RPC]
            .reshape(NT, P, NW)
            .transpose(1, 0, 2)
            .reshape(P, NT * NW)
        )
        in_maps.append({"wx": np.ascontiguousarray(np.concatenate([nx_rep, wr], axis=1))})

    if "nc" not in _cached:
        _cached["nc"] = _build_module()
    nc = _cached["nc"]

    r = run_bass_kernel_spmd(nc, in_maps, core_ids=list(range(NCORES)), **run_kwargs)

    outs = []
    for c in range(NCORES):
        m = r.results[c]["out"]            # [P, NT] uint32, nonzero = violation
        outs.append(m.T.reshape(RPC))      # row t*128+p within core
    vio = np.concatenate(outs)             # [8192]
    result = vio == 0
    if run_kwargs:
        return result, r
    return result
